# revision 5
# baseline (speedup 1.0000x reference)
"""Trainium2 Bass kernel for nn_Mesh_61924838474382 (gnn_message_passing).

Pipeline: CG solve of (I + 10*L) v = u on a 1024x1024 triangulated grid mesh,
then face centroids/normals + scatter-added vertex normals.

Strategy: the mesh from setup_inputs() is a fixed triangulated grid, so the
sparse matvec is a 6-neighbor stencil and the face/vertex normal phases are
regular shifted-window ops. Shard 128 grid rows per NeuronCore (partition dim
= grid row), keep all CG state SBUF-resident, exchange halo rows + dot
products with one AllGather per CG iteration (Chronopoulos-Gear one-sync CG).

kernel(u, faces, edges) accepts FULL inputs and returns the FULL outputs
(verts, centroid, fn, vn) like the reference. Structure of faces/edges is
verified at runtime; falls back to a scipy-based host path on mismatch.
"""
import numpy as np

G = 1024
NC = 8
ROWS = G // NC          # 128 grid rows per core
LAM = 10.0
ITERS = 64
PW = G + 2              # padded j width (zero col at 0 and G+1)
EPS = 1e-12

_CACHE = {}


# --------------------------------------------------------------------------
# Bass program
# --------------------------------------------------------------------------
def _build_program(iters):
    import concourse.bass as bass
    import concourse.bacc as bacc
    import concourse.tile as tile
    import concourse.mybir as mybir

    f32 = mybir.dt.float32
    i32 = mybir.dt.int32
    ALU = mybir.AluOpType

    nc = bacc.Bacc("TRN2", target_bir_lowering=False, debug=False,
                   num_devices=NC)

    # ---- per-core external inputs ----
    u_d = nc.dram_tensor("u", [ROWS, 3, PW], f32, kind="ExternalInput")
    dg_d = nc.dram_tensor("dg", [ROWS, 1, G], f32, kind="ExternalInput")
    rh0_d = nc.dram_tensor("rh0", [2, 3, PW], f32, kind="ExternalInput")
    m_d = nc.dram_tensor("m", [1, 2], f32, kind="ExternalInput")  # m_top, m_bot
    rmask_d = nc.dram_tensor("rmask", [ROWS, 1], f32, kind="ExternalInput")
    # AG rows: core c's contributions land at agout rows 2c, 2c+1.
    idxu_d = nc.dram_tensor("idxu", [2, 1], i32, kind="ExternalInput")  # [2(k-1)+1]*2
    idxd_d = nc.dram_tensor("idxd", [2, 1], i32, kind="ExternalInput")  # [2(k+1)]*2
    idxf_d = nc.dram_tensor("idxf", [2, 1], i32, kind="ExternalInput")  # [2(k-1), 2(k-1)+1]

    # ---- per-core external outputs ----
    o_verts = nc.dram_tensor("o_verts", [ROWS, 3, G], f32, kind="ExternalOutput")
    o_vn = nc.dram_tensor("o_vn", [ROWS, 3, G], f32, kind="ExternalOutput")
    o_cen1 = nc.dram_tensor("o_cen1", [ROWS, 3, G - 1], f32, kind="ExternalOutput")
    o_cen2 = nc.dram_tensor("o_cen2", [ROWS, 3, G - 1], f32, kind="ExternalOutput")
    o_fn1 = nc.dram_tensor("o_fn1", [ROWS, 3, G - 1], f32, kind="ExternalOutput")
    o_fn2 = nc.dram_tensor("o_fn2", [ROWS, 3, G - 1], f32, kind="ExternalOutput")

    # AG row layout (per row of the [2, RW] buffer):
    #   CG row 0: [gamma_p, delta_p, pad6, w row0 interior (3G)]
    #   CG row 1: [pad8,              w row127 interior (3G)]
    #   fn  row 0: [pad8, F1 row127 padded (3(G+1))]
    #   fn  row 1: [pad8, F2 row127 padded (3(G+1))]
    RW = 8 + 3 * (G + 1)
    W_LO, W_HI = 8, 8 + 3 * G
    FN_LO, FN_HI = 8, 8 + 3 * (G + 1)

    with tile.TileContext(nc) as tc:
        with (
            tc.tile_pool(name="sb", bufs=1) as sb,
            tc.tile_pool(name="ps", bufs=1, space="PSUM") as ps,
            tc.tile_pool(name="dram", bufs=1, space="DRAM") as dram,
        ):
            # ---- big tiles ----
            r = sb.tile([ROWS, 3, PW], f32, tag="r")
            U = sb.tile([ROWS, 3, PW], f32, tag="U")
            D = sb.tile([ROWS, 3, PW], f32, tag="D")
            w = sb.tile([ROWS, 3, G], f32, tag="w")
            p = sb.tile([ROWS, 3, G], f32, tag="p")
            s = sb.tile([ROWS, 3, G], f32, tag="s")
            x = sb.tile([ROWS, 3, G], f32, tag="x")
            dg = sb.tile([ROWS, 1, G], f32, tag="dg")
            scr = sb.tile([ROWS, 3, G], f32, tag="scr")   # dot out + x-upd tmp
            t1 = sb.tile([ROWS, 3, G + 1], f32, tag="t1")
            t2 = sb.tile([ROWS, 3, G + 1], f32, tag="t2")
            t3 = sb.tile([ROWS, 3, G + 1], f32, tag="t3")

            # ---- small tiles ----
            rHb = sb.tile([1, 3, PW], f32, tag="rHb")
            sHt = sb.tile([1, 3, PW], f32, tag="sHt")
            sHb = sb.tile([1, 3, PW], f32, tag="sHb")
            hgU = sb.tile([2, RW], f32, tag="hgU")
            hgD = sb.tile([2, RW], f32, tag="hgD")
            dots8 = sb.tile([1, 16], f32, tag="dots8")
            gsum = sb.tile([1, 2], f32, tag="gsum")
            sc = sb.tile([1, 16], f32, tag="sc")
            prs = sb.tile([1, 2], f32, tag="prs")
            accG = sb.tile([ROWS, 1], f32, tag="accG")
            accD = sb.tile([ROWS, 1], f32, tag="accD")
            onescol = sb.tile([ROWS, 1], f32, tag="onescol")
            onesrow = sb.tile([1, ROWS], f32, tag="onesrow")
            mT = sb.tile([1, 2], f32, tag="mT")
            rmask = sb.tile([ROWS, 1], f32, tag="rmask")
            idxu = sb.tile([2, 1], i32, tag="idxu")
            idxd = sb.tile([2, 1], i32, tag="idxd")
            idxf = sb.tile([2, 1], i32, tag="idxf")
            bcs = sb.tile([ROWS, 4], f32, tag="bcs")

            pr = ps.tile([1, 2], f32, tag="pr")
            bcp = ps.tile([ROWS, 4], f32, tag="bcp")

            agin = dram.tile([2, RW], f32)
            agout = dram.tile([2 * NC, RW], f32)

            # views
            rC = r[:, :, 1:1 + G]
            rHt = U[0:1, :, :]            # top halo row lives in U row 0
            tmpH = t2[0:1, :, 0:G]        # scratch halo row (t2 free then)
            dgb = dg[:, 0:1, :].broadcast_to([ROWS, 3, G])

            # ---- init ----
            nc.sync.dma_start(r[:], u_d[:])              # r0 = b (pre-padded)
            nc.sync.dma_start(dg[:], dg_d[:])
            nc.vector.memset(U[:], 0.0)
            nc.sync.dma_start(U[0:1, :, :], rh0_d[0:1, :, :])  # pre-masked host
            nc.sync.dma_start(rHb[:], rh0_d[1:2, :, :])
            nc.sync.dma_start(mT[:], m_d[:])
            nc.sync.dma_start(rmask[:], rmask_d[:])
            nc.sync.dma_start(idxu[:], idxu_d[:])
            nc.sync.dma_start(idxd[:], idxd_d[:])
            nc.sync.dma_start(idxf[:], idxf_d[:])
            nc.vector.memset(onescol[:], 1.0)
            nc.vector.memset(onesrow[:], 1.0)
            nc.vector.memset(x[:], 0.0)
            nc.vector.memset(p[:], 0.0)
            nc.vector.memset(s[:], 0.0)
            nc.vector.memset(sHt[:], 0.0)
            nc.vector.memset(sHb[:], 0.0)
            nc.vector.memset(D[:], 0.0)
            # sc slots: 0=beta 1=tmp 2=z 3=tmp 4=alpha 5=nalpha 8=rg_old 9=z_old
            nc.vector.memset(sc[:], 0.0)

            def shift_dma(dst, dst_lo, src, src_lo, n):
                CH = 16
                off = 0
                while off < n:
                    c = min(CH, n - off)
                    nc.sync.dma_start(
                        dst[dst_lo + off:dst_lo + off + c, :, :],
                        src[src_lo + off:src_lo + off + c, :, :])
                    off += c

            def fill_UD():
                shift_dma(U, 1, r, 0, ROWS - 1)
                shift_dma(D, 0, r, 1, ROWS - 1)
                nc.sync.dma_start(D[ROWS - 1:ROWS, :, :], rHb[:])

            fill_UD()

            for it in range(iters):
                # ---- matvec: w = dg*r - 10*N(r) ----
                nc.vector.tensor_add(t1[:, :, 0:G], r[:, :, 0:G], r[:, :, 2:2 + G])
                nc.vector.tensor_add(t2[:, :, 0:G], U[:, :, 1:1 + G], U[:, :, 2:2 + G])
                nc.gpsimd.tensor_add(t3[:, :, 0:G], D[:, :, 1:1 + G], D[:, :, 0:G])
                nc.vector.affine_mul_reduce(out=scr[:], accum_out=accG[:],
                                            in0=rC, in1=rC, scale=1.0, bias=0.0)
                nc.vector.tensor_add(t1[:, :, 0:G], t1[:, :, 0:G], t2[:, :, 0:G])
                nc.vector.tensor_mul(t2[:, :, 0:G], dgb, rC)
                nc.vector.tensor_add(t1[:, :, 0:G], t1[:, :, 0:G], t3[:, :, 0:G])
                nc.vector.affine_then_add(w[:], t1[:, :, 0:G], t2[:, :, 0:G],
                                          -LAM, 0.0)

                # ---- dots + C1 ----
                nc.vector.affine_mul_reduce(out=scr[:], accum_out=accD[:],
                                            in0=rC, in1=w[:], scale=1.0, bias=0.0)
                nc.tensor.matmul(pr[0:1, 0:1], onescol[:], accG[:],
                                 start=True, stop=True)
                nc.tensor.matmul(pr[0:1, 1:2], onescol[:], accD[:],
                                 start=True, stop=True)
                nc.vector.tensor_copy(prs[:], pr[0:1, :])
                nc.sync.dma_start(agin[0:1, 0:2], prs[:])
                nc.sync.dma_start(agin[0:1, W_LO:W_HI], w[0:1, :, :])
                nc.sync.dma_start(agin[1:2, W_LO:W_HI], w[ROWS - 1:ROWS, :, :])
                nc.gpsimd.collective_compute(
                    "AllGather", ALU.bypass,
                    replica_groups=[list(range(NC))],
                    ins=[agin.opt()], outs=[agout.opt()])
                nc.sync.dma_start(
                    dots8[0:1, :],
                    agout[:, 0:2].rearrange("(a b) c -> a b c", b=2)[:, 0, :])
                nc.gpsimd.indirect_dma_start(
                    hgU[:], None, agout[:, :],
                    bass.IndirectOffsetOnAxis(ap=idxu[:], axis=0))
                nc.gpsimd.indirect_dma_start(
                    hgD[:], None, agout[:, :],
                    bass.IndirectOffsetOnAxis(ap=idxd[:], axis=0))

                # ---- scalars ----
                nc.vector.tensor_reduce(
                    gsum[:],
                    dots8[0:1, :].rearrange("p (k c) -> p c k", k=NC),
                    axis=mybir.AxisListType.X, op=ALU.add)
                g_ = gsum[0:1, 0:1]
                d_ = gsum[0:1, 1:2]
                nc.vector.tensor_mul(sc[0:1, 0:1], g_, sc[0:1, 8:9])   # beta
                nc.vector.tensor_mul(sc[0:1, 1:2], sc[0:1, 0:1], sc[0:1, 9:10])
                nc.vector.tensor_mul(sc[0:1, 1:2], sc[0:1, 0:1], sc[0:1, 1:2])
                nc.vector.tensor_sub(sc[0:1, 2:3], d_, sc[0:1, 1:2])   # z
                nc.vector.reciprocal(sc[0:1, 3:4], sc[0:1, 2:3])
                nc.vector.tensor_mul(sc[0:1, 4:5], g_, sc[0:1, 3:4])   # alpha
                nc.vector.tensor_scalar_mul(sc[0:1, 5:6], sc[0:1, 4:5], -1.0)
                nc.vector.reciprocal(sc[0:1, 8:9], g_)                 # rg_old
                nc.vector.tensor_copy(sc[0:1, 9:10], sc[0:1, 2:3])     # z_old
                nc.tensor.matmul(bcp[:, 0:2], onesrow[:], sc[0:1, 4:6],
                                 start=True, stop=True)
                nc.tensor.matmul(bcp[:, 2:3], onesrow[:], sc[0:1, 0:1],
                                 start=True, stop=True)
                nc.vector.tensor_copy(bcs[:, 0:3], bcp[:, 0:3])

                al, nal, be = bcs[:, 0:1], bcs[:, 1:2], bcs[:, 2:3]
                be1, nal1 = sc[0:1, 0:1], sc[0:1, 5:6]

                # ---- vector updates ----
                nc.vector.affine_then_add(p[:], p[:], rC, be, 0.0)
                nc.vector.affine_then_add(s[:], s[:], w[:], be, 0.0)
                nc.gpsimd.tensor_scalar_mul(scr[:], p[:], al)
                nc.gpsimd.tensor_add(x[:], x[:], scr[:])
                nc.vector.affine_then_add(rC, s[:], rC, nal, 0.0)

                # ---- halo updates ----
                nc.vector.tensor_scalar_mul(tmpH, hgU[0:1, W_LO:W_HI],
                                            mT[0:1, 0:1])
                nc.vector.affine_then_add(sHt[:, :, 1:1 + G], sHt[:, :, 1:1 + G],
                                          tmpH, be1, 0.0)
                nc.vector.affine_then_add(rHt[:, :, 1:1 + G], sHt[:, :, 1:1 + G],
                                          rHt[:, :, 1:1 + G], nal1, 0.0)
                nc.vector.tensor_scalar_mul(tmpH, hgD[0:1, W_LO:W_HI],
                                            mT[0:1, 1:2])
                nc.vector.affine_then_add(sHb[:, :, 1:1 + G], sHb[:, :, 1:1 + G],
                                          tmpH, be1, 0.0)
                nc.vector.affine_then_add(rHb[:, :, 1:1 + G], sHb[:, :, 1:1 + G],
                                          rHb[:, :, 1:1 + G], nal1, 0.0)

                if it < iters - 1:
                    fill_UD()

            # ================= normals phase =================
            # x holds verts shard. Exchange x row0 (need core k+1's row0).
            nc.sync.dma_start(agin[0:1, W_LO:W_HI], x[0:1, :, :])
            nc.gpsimd.collective_compute(
                "AllGather", ALU.bypass,
                replica_groups=[list(range(NC))],
                ins=[agin.opt()], outs=[agout.opt()])
            nc.gpsimd.indirect_dma_start(
                hgD[:], None, agout[:, :],
                bass.IndirectOffsetOnAxis(ap=idxd[:], axis=0))

            nc.sync.dma_start(o_verts[:], x[:])
            v = r            # reuse r as padded verts
            vdn = U          # reuse U as down-shifted verts
            nc.vector.tensor_copy(v[:, :, 1:1 + G], x[:])
            shift_dma(vdn, 0, v, 1, ROWS - 1)
            nc.sync.dma_start(vdn[ROWS - 1:ROWS, :, 1:1 + G],
                              hgD[0:1, W_LO:W_HI])

            J = G - 1
            A = sb.tile([ROWS, 3, J], f32, tag="w")
            B = sb.tile([ROWS, 3, J], f32, tag="p")
            C3 = sb.tile([ROWS, 3, J], f32, tag="s")
            Q = sb.tile([ROWS, 1, J], f32, tag="scr")
            F1 = sb.tile([ROWS, 3, G + 1], f32, tag="t1")
            F2 = sb.tile([ROWS, 3, G + 1], f32, tag="t2")
            F1u = sb.tile([ROWS, 3, G + 1], f32, tag="t3")
            F2u = sb.tile([ROWS, 3, G + 1], f32, tag="D")
            VN = sb.tile([ROWS, 3, G], f32, tag="x")

            def face_set(va, vb, vc, o_cen, o_fn, Fdst):
                (ta, oa), (tb, ob), (tc_, oc) = va, vb, vc
                nc.vector.tensor_sub(A[:], tb[:, :, ob:ob + J], ta[:, :, oa:oa + J])
                nc.vector.tensor_sub(B[:], tc_[:, :, oc:oc + J], ta[:, :, oa:oa + J])
                for cd in range(3):
                    c1, c2 = (cd + 1) % 3, (cd + 2) % 3
                    nc.vector.tensor_mul(C3[:, cd:cd + 1, :],
                                         A[:, c1:c1 + 1, :], B[:, c2:c2 + 1, :])
                    nc.gpsimd.tensor_mul(Q[:],
                                         A[:, c2:c2 + 1, :], B[:, c1:c1 + 1, :])
                    nc.vector.tensor_sub(C3[:, cd:cd + 1, :],
                                         C3[:, cd:cd + 1, :], Q[:])
                nc.vector.tensor_mul(Q[:], C3[:, 0:1, :], C3[:, 0:1, :])
                nc.gpsimd.tensor_mul(A[:, 0:1, :], C3[:, 1:2, :], C3[:, 1:2, :])
                nc.vector.tensor_add(Q[:], Q[:], A[:, 0:1, :])
                nc.gpsimd.tensor_mul(A[:, 1:2, :], C3[:, 2:3, :], C3[:, 2:3, :])
                nc.vector.tensor_add(Q[:], Q[:], A[:, 1:2, :])
                nc.scalar.sqrt(Q[:], Q[:])
                nc.vector.tensor_scalar_max(Q[:], Q[:], float(EPS))
                nc.vector.reciprocal(Q[:], Q[:])
                for cd in range(3):
                    nc.vector.tensor_mul(Fdst[:, cd:cd + 1, 1:1 + J],
                                         C3[:, cd:cd + 1, :], Q[:])
                nc.sync.dma_start(o_fn[:], Fdst[:, :, 1:1 + J])
                nc.vector.tensor_add(A[:], ta[:, :, oa:oa + J], tb[:, :, ob:ob + J])
                nc.vector.tensor_add(A[:], A[:], tc_[:, :, oc:oc + J])
                nc.vector.tensor_scalar_mul(B[:], A[:],
                                            float(np.float32(1.0 / 3.0)))
                nc.sync.dma_start(o_cen[:], B[:])

            nc.vector.memset(F1[:], 0.0)
            nc.vector.memset(F2[:], 0.0)
            face_set((v, 1), (v, 2), (vdn, 1), o_cen1, o_fn1, F1)
            face_set((v, 2), (vdn, 2), (vdn, 1), o_cen2, o_fn2, F2)

            # zero face-row 127 on the last core (face row 1023 is fictitious)
            nc.vector.tensor_scalar_mul(F1[:], F1[:], rmask[:])
            nc.vector.tensor_scalar_mul(F2[:], F2[:], rmask[:])

            # exchange fn halo rows
            nc.sync.dma_start(agin[0:1, FN_LO:FN_HI], F1[ROWS - 1:ROWS, :, :])
            nc.sync.dma_start(agin[1:2, FN_LO:FN_HI], F2[ROWS - 1:ROWS, :, :])
            nc.gpsimd.collective_compute(
                "AllGather", ALU.bypass,
                replica_groups=[list(range(NC))],
                ins=[agin.opt()], outs=[agout.opt()])
            nc.gpsimd.indirect_dma_start(
                hgU[:], None, agout[:, :],
                bass.IndirectOffsetOnAxis(ap=idxf[:], axis=0))
            nc.sync.dma_start(hgD[0:1, :], hgU[1:2, :])

            shift_dma(F1u, 1, F1, 0, ROWS - 1)
            shift_dma(F2u, 1, F2, 0, ROWS - 1)
            nc.vector.tensor_copy(F1u[0:1, :, :], hgU[0:1, FN_LO:FN_HI])
            nc.vector.tensor_copy(F2u[0:1, :, :], hgD[0:1, FN_LO:FN_HI])

            nc.vector.tensor_add(VN[:], F1[:, :, 1:1 + G], F1[:, :, 0:G])
            nc.vector.tensor_add(VN[:], VN[:], F2[:, :, 0:G])
            nc.vector.tensor_add(VN[:], VN[:], F1u[:, :, 1:1 + G])
            nc.vector.tensor_add(VN[:], VN[:], F2u[:, :, 0:G])
            nc.vector.tensor_add(VN[:], VN[:], F2u[:, :, 1:1 + G])
            QV = sb.tile([ROWS, 1, G], f32, tag="hgU")
            QT = sb.tile([ROWS, 1, G], f32, tag="hgD")
            nc.vector.tensor_mul(QV[:], VN[:, 0:1, :], VN[:, 0:1, :])
            nc.vector.tensor_mul(QT[:], VN[:, 1:2, :], VN[:, 1:2, :])
            nc.vector.tensor_add(QV[:], QV[:], QT[:])
            nc.vector.tensor_mul(QT[:], VN[:, 2:3, :], VN[:, 2:3, :])
            nc.vector.tensor_add(QV[:], QV[:], QT[:])
            nc.scalar.sqrt(QV[:], QV[:])
            nc.vector.tensor_scalar_max(QV[:], QV[:], float(EPS))
            nc.vector.reciprocal(QV[:], QV[:])
            for cd in range(3):
                nc.vector.tensor_mul(VN[:, cd:cd + 1, :],
                                     VN[:, cd:cd + 1, :], QV[:])

            nc.sync.dma_start(o_vn[:], VN[:])

    nc.compile()
    return nc


# --------------------------------------------------------------------------
# Host-side driver
# --------------------------------------------------------------------------
def _grid_faces_edges():
    idx = np.arange(G * G, dtype=np.int64).reshape(G, G)
    v00 = idx[:-1, :-1].ravel(); v01 = idx[:-1, 1:].ravel()
    v10 = idx[1:, :-1].ravel(); v11 = idx[1:, 1:].ravel()
    f1 = np.stack([v00, v01, v10], axis=1)
    f2 = np.stack([v01, v11, v10], axis=1)
    return np.concatenate([f1, f2], axis=0)


def _check_grid(faces, edges):
    faces = np.asarray(faces)
    edges = np.asarray(edges)
    if faces.shape != (2 * (G - 1) * (G - 1), 3):
        return False
    if edges.shape != (2 * G * (G - 1) + (G - 1) * (G - 1), 2):
        return False
    rng = np.random.default_rng(0)
    fs = rng.integers(0, faces.shape[0], 2000)
    gf = _grid_faces_edges()
    if not np.array_equal(faces[fs], gf[fs]):
        return False
    es = rng.integers(0, edges.shape[0], 2000)
    e = edges[es]
    a, b = e[:, 0], e[:, 1]
    ai, aj = a // G, a % G
    bi, bj = b // G, b % G
    di, dj = bi - ai, bj - aj
    ok = ((di == 0) & (abs(dj) == 1)) | ((abs(di) == 1) & (dj == 0)) | \
         ((di == 1) & (dj == -1)) | ((di == -1) & (dj == 1))
    return bool(ok.all())


def _fallback(u, faces, edges):
    """Host scipy path mirroring the reference (slow, safety only)."""
    import scipy.sparse as sp
    u = np.asarray(u, np.float32)
    faces = np.asarray(faces)
    edges = np.asarray(edges)
    n = u.shape[0]
    src = np.concatenate([edges[:, 0], edges[:, 1]])
    dst = np.concatenate([edges[:, 1], edges[:, 0]])
    deg = np.bincount(src, minlength=n).astype(np.float32)
    W = sp.coo_matrix((np.ones(src.shape[0], np.float32), (src, dst)),
                      shape=(n, n)).tocsr()
    diag = (1.0 + LAM * deg).astype(np.float32)

    def matvec(xm):
        return diag[:, None] * xm - np.float32(LAM) * (W @ xm)

    x = np.zeros_like(u); rr = u.copy(); pp = rr.copy()
    gamma = np.float32((rr * rr).sum(dtype=np.float32))
    for _ in range(ITERS):
        Ap = matvec(pp).astype(np.float32)
        alpha = np.float32(gamma / np.float32((pp * Ap).sum(dtype=np.float32)))
        x = (x + alpha * pp).astype(np.float32)
        rr = (rr - alpha * Ap).astype(np.float32)
        gn = np.float32((rr * rr).sum(dtype=np.float32))
        beta = np.float32(gn / gamma); gamma = gn
        pp = (rr + beta * pp).astype(np.float32)
    verts = x
    tri = verts[faces]
    centroid = tri.mean(axis=-2).astype(np.float32)
    a = tri[:, 1] - tri[:, 0]
    b = tri[:, 2] - tri[:, 0]
    c = np.cross(a, b).astype(np.float32)
    fn = (c / np.maximum(np.linalg.norm(c, axis=-1, keepdims=True),
                         EPS)).astype(np.float32)
    vn = np.zeros((n, 3), np.float32)
    np.add.at(vn, faces.reshape(-1), np.repeat(fn, 3, axis=0))
    vn = (vn / np.maximum(np.linalg.norm(vn, axis=-1, keepdims=True),
                          EPS)).astype(np.float32)
    return verts, centroid, fn, vn


def kernel(u, faces, edges):
    import concourse.bass_utils as bass_utils

    u = np.asarray(u, dtype=np.float32)
    if not _check_grid(faces, edges):
        return _fallback(u, faces, edges)

    edges_np = np.asarray(edges)
    src = np.concatenate([edges_np[:, 0], edges_np[:, 1]])
    deg = np.bincount(src, minlength=G * G).astype(np.float32)
    diag = (1.0 + np.float32(LAM) * deg).reshape(G, G)

    ug = u.reshape(G, G, 3)
    in_maps = []
    for k in range(NC):
        lo = k * ROWS
        us = np.zeros((ROWS, 3, PW), np.float32)
        us[:, :, 1:1 + G] = ug[lo:lo + ROWS].transpose(0, 2, 1)
        dgs = np.ascontiguousarray(diag[lo:lo + ROWS, None, :])
        rh0 = np.zeros((2, 3, PW), np.float32)
        if k > 0:
            rh0[0, :, 1:1 + G] = ug[lo - 1].T
        if k < NC - 1:
            rh0[1, :, 1:1 + G] = ug[lo + ROWS].T
        m = np.array([[0.0 if k == 0 else 1.0,
                       0.0 if k == NC - 1 else 1.0]], np.float32)
        rmask = np.ones((ROWS, 1), np.float32)
        if k == NC - 1:
            rmask[ROWS - 1, 0] = 0.0
        idxu = np.array([[2 * ((k - 1) % NC) + 1]] * 2, np.int32)
        idxd = np.array([[2 * ((k + 1) % NC)]] * 2, np.int32)
        idxf = np.array([[2 * ((k - 1) % NC)], [2 * ((k - 1) % NC) + 1]],
                        np.int32)
        in_maps.append({"u": us, "dg": dgs, "rh0": rh0, "m": m,
                        "rmask": rmask, "idxu": idxu, "idxd": idxd,
                        "idxf": idxf})

    iters = int(_CACHE.get("iters_override", ITERS))
    key = ("nc", iters)
    if key not in _CACHE:
        _CACHE[key] = _build_program(iters)
    nc = _CACHE[key]

    res = bass_utils.run_bass_kernel_spmd(nc, in_maps,
                                          core_ids=list(range(NC)))
    rs = res.results

    V = G * G
    F2c = (G - 1) * (G - 1)
    verts = np.empty((G, G, 3), np.float32)
    vn = np.empty((G, G, 3), np.float32)
    cen = np.empty((2 * F2c, 3), np.float32)
    fn = np.empty((2 * F2c, 3), np.float32)
    cen1 = cen[:F2c].reshape(G - 1, G - 1, 3)
    cen2 = cen[F2c:].reshape(G - 1, G - 1, 3)
    fn1 = fn[:F2c].reshape(G - 1, G - 1, 3)
    fn2 = fn[F2c:].reshape(G - 1, G - 1, 3)
    for k in range(NC):
        lo = k * ROWS
        rk = rs[k]
        verts[lo:lo + ROWS] = rk["o_verts"].transpose(0, 2, 1)
        vn[lo:lo + ROWS] = rk["o_vn"].transpose(0, 2, 1)
        hi = min(lo + ROWS, G - 1)
        nrow = hi - lo
        if nrow > 0:
            cen1[lo:hi] = rk["o_cen1"][:nrow].transpose(0, 2, 1)
            cen2[lo:hi] = rk["o_cen2"][:nrow].transpose(0, 2, 1)
            fn1[lo:hi] = rk["o_fn1"][:nrow].transpose(0, 2, 1)
            fn2[lo:hi] = rk["o_fn2"][:nrow].transpose(0, 2, 1)
    return (verts.reshape(V, 3), cen, fn, vn.reshape(V, 3))


# revision 9
# speedup vs baseline: 12.8024x; 12.8024x over previous
"""Trainium2 Bass kernel for nn_Mesh_61924838474382 (gnn_message_passing).

Pipeline: CG solve of (I + 10*L) v = u on a 1024x1024 triangulated grid mesh,
then face centroids/normals + scatter-added vertex normals.

Strategy: the mesh from setup_inputs() is a fixed triangulated grid, so the
sparse matvec is a 6-neighbor stencil and the face/vertex normal phases are
regular shifted-window ops. Shard 128 grid rows per NeuronCore (partition dim
= grid row), keep all CG state SBUF-resident, exchange halo rows + dot
products with one AllGather per CG iteration (Chronopoulos-Gear one-sync CG).

kernel(u, faces, edges) accepts FULL inputs and returns the FULL outputs
(verts, centroid, fn, vn) like the reference. Structure of faces/edges is
verified at runtime; falls back to a scipy-based host path on mismatch.
"""
import numpy as np

G = 1024
NC = 8
ROWS = G // NC          # 128 grid rows per core
LAM = 10.0
ITERS = 64
PW = G + 2              # padded j width (zero col at 0 and G+1)
EPS = 1e-12

_CACHE = {}


# --------------------------------------------------------------------------
# Bass program
# --------------------------------------------------------------------------
def _build_program(iters):
    import concourse.bass as bass
    import concourse.bacc as bacc
    import concourse.tile as tile
    import concourse.mybir as mybir

    f32 = mybir.dt.float32
    i32 = mybir.dt.int32
    ALU = mybir.AluOpType

    nc = bacc.Bacc("TRN2", target_bir_lowering=False, debug=False,
                   num_devices=NC)

    # ---- per-core external inputs ----
    u_d = nc.dram_tensor("u", [ROWS, 3, PW], f32, kind="ExternalInput")
    dg_d = nc.dram_tensor("dg", [ROWS, 1, G], f32, kind="ExternalInput")
    rh0_d = nc.dram_tensor("rh0", [2, 3, PW], f32, kind="ExternalInput")
    m_d = nc.dram_tensor("m", [1, 2], f32, kind="ExternalInput")  # m_top, m_bot
    rmask_d = nc.dram_tensor("rmask", [ROWS, 1], f32, kind="ExternalInput")
    # AG rows: core c's contributions land at agout rows 2c, 2c+1.
    idxu_d = nc.dram_tensor("idxu", [2, 1], i32, kind="ExternalInput")  # [2(k-1)+1]*2
    idxd_d = nc.dram_tensor("idxd", [2, 1], i32, kind="ExternalInput")  # [2(k+1)]*2
    idxf_d = nc.dram_tensor("idxf", [2, 1], i32, kind="ExternalInput")  # [2(k-1), 2(k-1)+1]

    # ---- per-core external outputs ----
    o_verts = nc.dram_tensor("o_verts", [ROWS, 3, G], f32, kind="ExternalOutput")
    o_vn = nc.dram_tensor("o_vn", [ROWS, 3, G], f32, kind="ExternalOutput")
    o_cen1 = nc.dram_tensor("o_cen1", [ROWS, 3, G - 1], f32, kind="ExternalOutput")
    o_cen2 = nc.dram_tensor("o_cen2", [ROWS, 3, G - 1], f32, kind="ExternalOutput")
    o_fn1 = nc.dram_tensor("o_fn1", [ROWS, 3, G - 1], f32, kind="ExternalOutput")
    o_fn2 = nc.dram_tensor("o_fn2", [ROWS, 3, G - 1], f32, kind="ExternalOutput")

    # AG row layout (per row of the [2, RW] buffer):
    #   CG row 0: [gamma_p, delta_p, pad6, w row0 interior (3G)]
    #   CG row 1: [pad8,              w row127 interior (3G)]
    #   fn  row 0: [pad8, F1 row127 padded (3(G+1))]
    #   fn  row 1: [pad8, F2 row127 padded (3(G+1))]
    RW = 8 + 3 * (G + 1)
    W_LO, W_HI = 8, 8 + 3 * G
    FN_LO, FN_HI = 8, 8 + 3 * (G + 1)

    with tile.TileContext(nc) as tc:
        with (
            tc.tile_pool(name="sb", bufs=1) as sb,
            tc.tile_pool(name="ps", bufs=1, space="PSUM") as ps,
            tc.tile_pool(name="dram", bufs=1, space="DRAM") as dram,
        ):
            # ---- big tiles ----
            r = sb.tile([ROWS, 3, PW], f32, tag="r")
            U = sb.tile([ROWS, 3, PW], f32, tag="U")
            D = sb.tile([ROWS, 3, PW], f32, tag="D")
            w = sb.tile([ROWS, 3, G], f32, tag="w")
            p = sb.tile([ROWS, 3, G], f32, tag="p")
            s = sb.tile([ROWS, 3, G], f32, tag="s")
            x = sb.tile([ROWS, 3, G], f32, tag="x")
            dg = sb.tile([ROWS, 1, G], f32, tag="dg")
            scr = sb.tile([ROWS, 3, G], f32, tag="scr")   # dot out + x-upd tmp
            t1 = sb.tile([ROWS, 3, G + 1], f32, tag="t1")
            t2 = sb.tile([ROWS, 3, G + 1], f32, tag="t2")
            t3 = sb.tile([ROWS, 3, G + 1], f32, tag="t3")

            # ---- small tiles ----
            rHb = sb.tile([1, 3, PW], f32, tag="rHb")
            sHt = sb.tile([1, 3, PW], f32, tag="sHt")
            sHb = sb.tile([1, 3, PW], f32, tag="sHb")
            hgU = sb.tile([2, RW], f32, tag="hgU")
            hgD = sb.tile([2, RW], f32, tag="hgD")
            dots8 = sb.tile([1, 16], f32, tag="dots8")
            gsum = sb.tile([1, 2], f32, tag="gsum")
            sc = sb.tile([1, 16], f32, tag="sc")
            prs = sb.tile([1, 2], f32, tag="prs")
            accG = sb.tile([ROWS, 1], f32, tag="accG")
            accD = sb.tile([ROWS, 1], f32, tag="accD")
            onescol = sb.tile([ROWS, 1], f32, tag="onescol")
            onesrow = sb.tile([1, ROWS], f32, tag="onesrow")
            mT = sb.tile([1, 2], f32, tag="mT")
            rmask = sb.tile([ROWS, 1], f32, tag="rmask")
            idxu = sb.tile([2, 1], i32, tag="idxu")
            idxd = sb.tile([2, 1], i32, tag="idxd")
            idxf = sb.tile([2, 1], i32, tag="idxf")
            bcs = sb.tile([ROWS, 4], f32, tag="bcs")

            pr = ps.tile([1, 2], f32, tag="pr")
            bcp = ps.tile([ROWS, 4], f32, tag="bcp")

            agin = dram.tile([2, RW], f32)
            agout = dram.tile([2 * NC, RW], f32)

            # views
            rC = r[:, :, 1:1 + G]
            rHt = U[0:1, :, :]            # top halo row lives in U row 0
            tmpH = t2[0:1, :, 0:G]        # scratch halo row (t2 free then)
            dgb = dg[:, 0:1, :].broadcast_to([ROWS, 3, G])

            # ---- init ----
            nc.sync.dma_start(r[:], u_d[:])              # r0 = b (pre-padded)
            nc.sync.dma_start(dg[:], dg_d[:])
            nc.vector.memset(U[:], 0.0)
            nc.sync.dma_start(U[0:1, :, :], rh0_d[0:1, :, :])  # pre-masked host
            nc.sync.dma_start(rHb[:], rh0_d[1:2, :, :])
            nc.sync.dma_start(mT[:], m_d[:])
            nc.sync.dma_start(rmask[:], rmask_d[:])
            nc.sync.dma_start(idxu[:], idxu_d[:])
            nc.sync.dma_start(idxd[:], idxd_d[:])
            nc.sync.dma_start(idxf[:], idxf_d[:])
            nc.vector.memset(onescol[:], 1.0)
            nc.vector.memset(onesrow[:], 1.0)
            nc.vector.memset(x[:], 0.0)
            nc.vector.memset(p[:], 0.0)
            nc.vector.memset(s[:], 0.0)
            nc.vector.memset(sHt[:], 0.0)
            nc.vector.memset(sHb[:], 0.0)
            nc.vector.memset(D[:], 0.0)
            # sc slots: 0=beta 1=tmp 2=z 3=tmp 4=alpha 5=nalpha 8=rg_old 9=z_old
            nc.vector.memset(sc[:], 0.0)

            def shift_dma(dst, dst_lo, src, src_lo, n, phase=0):
                CH = 16
                off = 0
                i = phase
                while off < n:
                    c = min(CH, n - off)
                    eng = nc.sync if (i % 2 == 0) else nc.scalar
                    eng.dma_start(
                        dst[dst_lo + off:dst_lo + off + c, :, :],
                        src[src_lo + off:src_lo + off + c, :, :])
                    off += c
                    i += 1

            def fill_UD():
                shift_dma(U, 1, r, 0, ROWS - 1, 0)
                shift_dma(D, 0, r, 1, ROWS - 1, 1)
                nc.scalar.dma_start(D[ROWS - 1:ROWS, :, :], rHb[:])

            fill_UD()

            for it in range(iters):
                # ---- matvec: w = dg*r - 10*N(r) ----
                nc.vector.tensor_add(t1[:, :, 0:G], r[:, :, 0:G], r[:, :, 2:2 + G])
                nc.vector.tensor_add(t2[:, :, 0:G], U[:, :, 1:1 + G], U[:, :, 2:2 + G])
                nc.gpsimd.tensor_add(t3[:, :, 0:G], D[:, :, 1:1 + G], D[:, :, 0:G])
                nc.vector.affine_mul_reduce(out=scr[:], accum_out=accG[:],
                                            in0=rC, in1=rC, scale=1.0, bias=0.0)
                nc.vector.tensor_add(t1[:, :, 0:G], t1[:, :, 0:G], t2[:, :, 0:G])
                nc.vector.tensor_mul(t2[:, :, 0:G], dgb, rC)
                nc.vector.tensor_add(t1[:, :, 0:G], t1[:, :, 0:G], t3[:, :, 0:G])
                nc.vector.affine_then_add(w[:], t1[:, :, 0:G], t2[:, :, 0:G],
                                          -LAM, 0.0)

                # ---- dots + C1 ----
                nc.vector.affine_mul_reduce(out=scr[:], accum_out=accD[:],
                                            in0=rC, in1=w[:], scale=1.0, bias=0.0)
                nc.tensor.matmul(pr[0:1, 0:1], onescol[:], accG[:],
                                 start=True, stop=True)
                nc.tensor.matmul(pr[0:1, 1:2], onescol[:], accD[:],
                                 start=True, stop=True)
                nc.vector.tensor_copy(prs[:], pr[0:1, :])
                nc.sync.dma_start(agin[0:1, 0:2], prs[:])
                nc.sync.dma_start(agin[0:1, W_LO:W_HI], w[0:1, :, :])
                nc.sync.dma_start(agin[1:2, W_LO:W_HI], w[ROWS - 1:ROWS, :, :])
                nc.gpsimd.collective_compute(
                    "AllGather", ALU.bypass,
                    replica_groups=[list(range(NC))],
                    ins=[agin.opt()], outs=[agout.opt()])
                nc.sync.dma_start(
                    dots8[0:1, :],
                    agout[:, 0:2].rearrange("(a b) c -> a b c", b=2)[:, 0, :])
                nc.gpsimd.indirect_dma_start(
                    hgU[:], None, agout[:, :],
                    bass.IndirectOffsetOnAxis(ap=idxu[:], axis=0))
                nc.gpsimd.indirect_dma_start(
                    hgD[:], None, agout[:, :],
                    bass.IndirectOffsetOnAxis(ap=idxd[:], axis=0))

                # ---- scalars ----
                nc.vector.tensor_reduce(
                    gsum[:],
                    dots8[0:1, :].rearrange("p (k c) -> p c k", k=NC),
                    axis=mybir.AxisListType.X, op=ALU.add)
                g_ = gsum[0:1, 0:1]
                d_ = gsum[0:1, 1:2]
                nc.vector.tensor_mul(sc[0:1, 0:1], g_, sc[0:1, 8:9])   # beta
                nc.vector.tensor_mul(sc[0:1, 1:2], sc[0:1, 0:1], sc[0:1, 9:10])
                nc.vector.tensor_mul(sc[0:1, 1:2], sc[0:1, 0:1], sc[0:1, 1:2])
                nc.vector.tensor_sub(sc[0:1, 2:3], d_, sc[0:1, 1:2])   # z
                nc.vector.reciprocal(sc[0:1, 3:4], sc[0:1, 2:3])
                nc.vector.tensor_mul(sc[0:1, 4:5], g_, sc[0:1, 3:4])   # alpha
                nc.vector.tensor_scalar_mul(sc[0:1, 5:6], sc[0:1, 4:5], -1.0)
                nc.vector.reciprocal(sc[0:1, 8:9], g_)                 # rg_old
                nc.vector.tensor_copy(sc[0:1, 9:10], sc[0:1, 2:3])     # z_old
                nc.tensor.matmul(bcp[:, 0:2], onesrow[:], sc[0:1, 4:6],
                                 start=True, stop=True)
                nc.tensor.matmul(bcp[:, 2:3], onesrow[:], sc[0:1, 0:1],
                                 start=True, stop=True)
                nc.vector.tensor_copy(bcs[:, 0:3], bcp[:, 0:3])

                al, nal, be = bcs[:, 0:1], bcs[:, 1:2], bcs[:, 2:3]
                be1, nal1 = sc[0:1, 0:1], sc[0:1, 5:6]

                # ---- vector updates ----
                nc.vector.affine_then_add(p[:], p[:], rC, be, 0.0)
                nc.vector.affine_then_add(s[:], s[:], w[:], be, 0.0)
                nc.gpsimd.tensor_scalar_mul(scr[:], p[:], al)
                nc.gpsimd.tensor_add(x[:], x[:], scr[:])
                nc.vector.affine_then_add(rC, s[:], rC, nal, 0.0)

                # ---- halo updates ----
                nc.vector.tensor_scalar_mul(tmpH, hgU[0:1, W_LO:W_HI],
                                            mT[0:1, 0:1])
                nc.vector.affine_then_add(sHt[:, :, 1:1 + G], sHt[:, :, 1:1 + G],
                                          tmpH, be1, 0.0)
                nc.vector.affine_then_add(rHt[:, :, 1:1 + G], sHt[:, :, 1:1 + G],
                                          rHt[:, :, 1:1 + G], nal1, 0.0)
                nc.vector.tensor_scalar_mul(tmpH, hgD[0:1, W_LO:W_HI],
                                            mT[0:1, 1:2])
                nc.vector.affine_then_add(sHb[:, :, 1:1 + G], sHb[:, :, 1:1 + G],
                                          tmpH, be1, 0.0)
                nc.vector.affine_then_add(rHb[:, :, 1:1 + G], sHb[:, :, 1:1 + G],
                                          rHb[:, :, 1:1 + G], nal1, 0.0)

                if it < iters - 1:
                    fill_UD()

            # ================= normals phase =================
            # x holds verts shard. Exchange x row0 (need core k+1's row0).
            nc.sync.dma_start(agin[0:1, W_LO:W_HI], x[0:1, :, :])
            nc.gpsimd.collective_compute(
                "AllGather", ALU.bypass,
                replica_groups=[list(range(NC))],
                ins=[agin.opt()], outs=[agout.opt()])
            nc.gpsimd.indirect_dma_start(
                hgD[:], None, agout[:, :],
                bass.IndirectOffsetOnAxis(ap=idxd[:], axis=0))

            nc.sync.dma_start(o_verts[:], x[:])
            v = r            # reuse r as padded verts
            vdn = U          # reuse U as down-shifted verts
            nc.vector.tensor_copy(v[:, :, 1:1 + G], x[:])
            shift_dma(vdn, 0, v, 1, ROWS - 1)
            nc.sync.dma_start(vdn[ROWS - 1:ROWS, :, 1:1 + G],
                              hgD[0:1, W_LO:W_HI])

            J = G - 1
            A = sb.tile([ROWS, 3, J], f32, tag="w")
            B = sb.tile([ROWS, 3, J], f32, tag="p")
            C3 = sb.tile([ROWS, 3, J], f32, tag="s")
            Q = sb.tile([ROWS, 1, J], f32, tag="scr")
            F1 = sb.tile([ROWS, 3, G + 1], f32, tag="t1")
            F2 = sb.tile([ROWS, 3, G + 1], f32, tag="t2")
            F1u = sb.tile([ROWS, 3, G + 1], f32, tag="t3")
            F2u = sb.tile([ROWS, 3, G + 1], f32, tag="D")
            VN = sb.tile([ROWS, 3, G], f32, tag="x")

            def face_set(va, vb, vc, o_cen, o_fn, Fdst):
                (ta, oa), (tb, ob), (tc_, oc) = va, vb, vc
                nc.vector.tensor_sub(A[:], tb[:, :, ob:ob + J], ta[:, :, oa:oa + J])
                nc.vector.tensor_sub(B[:], tc_[:, :, oc:oc + J], ta[:, :, oa:oa + J])
                for cd in range(3):
                    c1, c2 = (cd + 1) % 3, (cd + 2) % 3
                    nc.vector.tensor_mul(C3[:, cd:cd + 1, :],
                                         A[:, c1:c1 + 1, :], B[:, c2:c2 + 1, :])
                    nc.gpsimd.tensor_mul(Q[:],
                                         A[:, c2:c2 + 1, :], B[:, c1:c1 + 1, :])
                    nc.vector.tensor_sub(C3[:, cd:cd + 1, :],
                                         C3[:, cd:cd + 1, :], Q[:])
                nc.vector.tensor_mul(Q[:], C3[:, 0:1, :], C3[:, 0:1, :])
                nc.gpsimd.tensor_mul(A[:, 0:1, :], C3[:, 1:2, :], C3[:, 1:2, :])
                nc.vector.tensor_add(Q[:], Q[:], A[:, 0:1, :])
                nc.gpsimd.tensor_mul(A[:, 1:2, :], C3[:, 2:3, :], C3[:, 2:3, :])
                nc.vector.tensor_add(Q[:], Q[:], A[:, 1:2, :])
                nc.scalar.sqrt(Q[:], Q[:])
                nc.vector.tensor_scalar_max(Q[:], Q[:], float(EPS))
                nc.vector.reciprocal(Q[:], Q[:])
                for cd in range(3):
                    nc.vector.tensor_mul(Fdst[:, cd:cd + 1, 1:1 + J],
                                         C3[:, cd:cd + 1, :], Q[:])
                nc.sync.dma_start(o_fn[:], Fdst[:, :, 1:1 + J])
                nc.vector.tensor_add(A[:], ta[:, :, oa:oa + J], tb[:, :, ob:ob + J])
                nc.vector.tensor_add(A[:], A[:], tc_[:, :, oc:oc + J])
                nc.vector.tensor_scalar_mul(B[:], A[:],
                                            float(np.float32(1.0 / 3.0)))
                nc.sync.dma_start(o_cen[:], B[:])

            nc.vector.memset(F1[:], 0.0)
            nc.vector.memset(F2[:], 0.0)
            face_set((v, 1), (v, 2), (vdn, 1), o_cen1, o_fn1, F1)
            face_set((v, 2), (vdn, 2), (vdn, 1), o_cen2, o_fn2, F2)

            # zero face-row 127 on the last core (face row 1023 is fictitious)
            nc.vector.tensor_scalar_mul(F1[:], F1[:], rmask[:])
            nc.vector.tensor_scalar_mul(F2[:], F2[:], rmask[:])

            # exchange fn halo rows
            nc.sync.dma_start(agin[0:1, FN_LO:FN_HI], F1[ROWS - 1:ROWS, :, :])
            nc.sync.dma_start(agin[1:2, FN_LO:FN_HI], F2[ROWS - 1:ROWS, :, :])
            nc.gpsimd.collective_compute(
                "AllGather", ALU.bypass,
                replica_groups=[list(range(NC))],
                ins=[agin.opt()], outs=[agout.opt()])
            nc.gpsimd.indirect_dma_start(
                hgU[:], None, agout[:, :],
                bass.IndirectOffsetOnAxis(ap=idxf[:], axis=0))
            nc.sync.dma_start(hgD[0:1, :], hgU[1:2, :])

            shift_dma(F1u, 1, F1, 0, ROWS - 1)
            shift_dma(F2u, 1, F2, 0, ROWS - 1)
            nc.vector.tensor_copy(F1u[0:1, :, :], hgU[0:1, FN_LO:FN_HI])
            nc.vector.tensor_copy(F2u[0:1, :, :], hgD[0:1, FN_LO:FN_HI])

            nc.vector.tensor_add(VN[:], F1[:, :, 1:1 + G], F1[:, :, 0:G])
            nc.vector.tensor_add(VN[:], VN[:], F2[:, :, 0:G])
            nc.vector.tensor_add(VN[:], VN[:], F1u[:, :, 1:1 + G])
            nc.vector.tensor_add(VN[:], VN[:], F2u[:, :, 0:G])
            nc.vector.tensor_add(VN[:], VN[:], F2u[:, :, 1:1 + G])
            QV = sb.tile([ROWS, 1, G], f32, tag="hgU")
            QT = sb.tile([ROWS, 1, G], f32, tag="hgD")
            nc.vector.tensor_mul(QV[:], VN[:, 0:1, :], VN[:, 0:1, :])
            nc.vector.tensor_mul(QT[:], VN[:, 1:2, :], VN[:, 1:2, :])
            nc.vector.tensor_add(QV[:], QV[:], QT[:])
            nc.vector.tensor_mul(QT[:], VN[:, 2:3, :], VN[:, 2:3, :])
            nc.vector.tensor_add(QV[:], QV[:], QT[:])
            nc.scalar.sqrt(QV[:], QV[:])
            nc.vector.tensor_scalar_max(QV[:], QV[:], float(EPS))
            nc.vector.reciprocal(QV[:], QV[:])
            for cd in range(3):
                nc.vector.tensor_mul(VN[:, cd:cd + 1, :],
                                     VN[:, cd:cd + 1, :], QV[:])

            nc.sync.dma_start(o_vn[:], VN[:])

    nc.compile()
    return nc


# --------------------------------------------------------------------------
# Host-side driver
# --------------------------------------------------------------------------
def _grid_faces_edges():
    idx = np.arange(G * G, dtype=np.int64).reshape(G, G)
    v00 = idx[:-1, :-1].ravel(); v01 = idx[:-1, 1:].ravel()
    v10 = idx[1:, :-1].ravel(); v11 = idx[1:, 1:].ravel()
    f1 = np.stack([v00, v01, v10], axis=1)
    f2 = np.stack([v01, v11, v10], axis=1)
    return np.concatenate([f1, f2], axis=0)


def _check_grid(faces, edges):
    faces = np.asarray(faces)
    edges = np.asarray(edges)
    if faces.shape != (2 * (G - 1) * (G - 1), 3):
        return False
    if edges.shape != (2 * G * (G - 1) + (G - 1) * (G - 1), 2):
        return False
    rng = np.random.default_rng(0)
    fs = rng.integers(0, faces.shape[0], 2000)
    gf = _grid_faces_edges()
    if not np.array_equal(faces[fs], gf[fs]):
        return False
    es = rng.integers(0, edges.shape[0], 2000)
    e = edges[es]
    a, b = e[:, 0], e[:, 1]
    ai, aj = a // G, a % G
    bi, bj = b // G, b % G
    di, dj = bi - ai, bj - aj
    ok = ((di == 0) & (abs(dj) == 1)) | ((abs(di) == 1) & (dj == 0)) | \
         ((di == 1) & (dj == -1)) | ((di == -1) & (dj == 1))
    return bool(ok.all())


def _fallback(u, faces, edges):
    """Host scipy path mirroring the reference (slow, safety only)."""
    import scipy.sparse as sp
    u = np.asarray(u, np.float32)
    faces = np.asarray(faces)
    edges = np.asarray(edges)
    n = u.shape[0]
    src = np.concatenate([edges[:, 0], edges[:, 1]])
    dst = np.concatenate([edges[:, 1], edges[:, 0]])
    deg = np.bincount(src, minlength=n).astype(np.float32)
    W = sp.coo_matrix((np.ones(src.shape[0], np.float32), (src, dst)),
                      shape=(n, n)).tocsr()
    diag = (1.0 + LAM * deg).astype(np.float32)

    def matvec(xm):
        return diag[:, None] * xm - np.float32(LAM) * (W @ xm)

    x = np.zeros_like(u); rr = u.copy(); pp = rr.copy()
    gamma = np.float32((rr * rr).sum(dtype=np.float32))
    for _ in range(ITERS):
        Ap = matvec(pp).astype(np.float32)
        alpha = np.float32(gamma / np.float32((pp * Ap).sum(dtype=np.float32)))
        x = (x + alpha * pp).astype(np.float32)
        rr = (rr - alpha * Ap).astype(np.float32)
        gn = np.float32((rr * rr).sum(dtype=np.float32))
        beta = np.float32(gn / gamma); gamma = gn
        pp = (rr + beta * pp).astype(np.float32)
    verts = x
    tri = verts[faces]
    centroid = tri.mean(axis=-2).astype(np.float32)
    a = tri[:, 1] - tri[:, 0]
    b = tri[:, 2] - tri[:, 0]
    c = np.cross(a, b).astype(np.float32)
    fn = (c / np.maximum(np.linalg.norm(c, axis=-1, keepdims=True),
                         EPS)).astype(np.float32)
    vn = np.zeros((n, 3), np.float32)
    np.add.at(vn, faces.reshape(-1), np.repeat(fn, 3, axis=0))
    vn = (vn / np.maximum(np.linalg.norm(vn, axis=-1, keepdims=True),
                          EPS)).astype(np.float32)
    return verts, centroid, fn, vn


def kernel(u, faces, edges):
    import concourse.bass_utils as bass_utils

    u = np.asarray(u, dtype=np.float32)
    if not _check_grid(faces, edges):
        return _fallback(u, faces, edges)

    edges_np = np.asarray(edges)
    src = np.concatenate([edges_np[:, 0], edges_np[:, 1]])
    deg = np.bincount(src, minlength=G * G).astype(np.float32)
    diag = (1.0 + np.float32(LAM) * deg).reshape(G, G)

    ug = u.reshape(G, G, 3)
    in_maps = []
    for k in range(NC):
        lo = k * ROWS
        us = np.zeros((ROWS, 3, PW), np.float32)
        us[:, :, 1:1 + G] = ug[lo:lo + ROWS].transpose(0, 2, 1)
        dgs = np.ascontiguousarray(diag[lo:lo + ROWS, None, :])
        rh0 = np.zeros((2, 3, PW), np.float32)
        if k > 0:
            rh0[0, :, 1:1 + G] = ug[lo - 1].T
        if k < NC - 1:
            rh0[1, :, 1:1 + G] = ug[lo + ROWS].T
        m = np.array([[0.0 if k == 0 else 1.0,
                       0.0 if k == NC - 1 else 1.0]], np.float32)
        rmask = np.ones((ROWS, 1), np.float32)
        if k == NC - 1:
            rmask[ROWS - 1, 0] = 0.0
        idxu = np.array([[2 * ((k - 1) % NC) + 1]] * 2, np.int32)
        idxd = np.array([[2 * ((k + 1) % NC)]] * 2, np.int32)
        idxf = np.array([[2 * ((k - 1) % NC)], [2 * ((k - 1) % NC) + 1]],
                        np.int32)
        in_maps.append({"u": us, "dg": dgs, "rh0": rh0, "m": m,
                        "rmask": rmask, "idxu": idxu, "idxd": idxd,
                        "idxf": idxf})

    iters = int(_CACHE.get("iters_override", ITERS))
    key = ("nc", iters)
    if key not in _CACHE:
        _CACHE[key] = _build_program(iters)
    nc = _CACHE[key]

    res = bass_utils.run_bass_kernel_spmd(nc, in_maps,
                                          core_ids=list(range(NC)))
    rs = res.results

    V = G * G
    F2c = (G - 1) * (G - 1)
    verts = np.empty((G, G, 3), np.float32)
    vn = np.empty((G, G, 3), np.float32)
    cen = np.empty((2 * F2c, 3), np.float32)
    fn = np.empty((2 * F2c, 3), np.float32)
    cen1 = cen[:F2c].reshape(G - 1, G - 1, 3)
    cen2 = cen[F2c:].reshape(G - 1, G - 1, 3)
    fn1 = fn[:F2c].reshape(G - 1, G - 1, 3)
    fn2 = fn[F2c:].reshape(G - 1, G - 1, 3)
    for k in range(NC):
        lo = k * ROWS
        rk = rs[k]
        verts[lo:lo + ROWS] = rk["o_verts"].transpose(0, 2, 1)
        vn[lo:lo + ROWS] = rk["o_vn"].transpose(0, 2, 1)
        hi = min(lo + ROWS, G - 1)
        nrow = hi - lo
        if nrow > 0:
            cen1[lo:hi] = rk["o_cen1"][:nrow].transpose(0, 2, 1)
            cen2[lo:hi] = rk["o_cen2"][:nrow].transpose(0, 2, 1)
            fn1[lo:hi] = rk["o_fn1"][:nrow].transpose(0, 2, 1)
            fn2[lo:hi] = rk["o_fn2"][:nrow].transpose(0, 2, 1)
    return (verts.reshape(V, 3), cen, fn, vn.reshape(V, 3))
